# revision 16
# baseline (speedup 1.0000x reference)
import sys

sys.path.insert(0, "/opt/trn_rl_repo")

import numpy as np
import ml_dtypes

import concourse.bass as bass
import concourse.mybir as mybir
import concourse.tile as tile
from concourse import bacc
from concourse.bass_utils import run_bass_kernel_spmd

F32 = mybir.dt.float32
BF16 = mybir.dt.bfloat16
AF = mybir.ActivationFunctionType
OP = mybir.AluOpType
BF = ml_dtypes.bfloat16

D = 640
DCH = 5
H = 10
DH = 64
T = 1024
W = 128
NW = T // W
NB = 4096
NBC = NB // 128
HID = 4 * D
HCH = HID // 128
N_CORES = 8
EPS = 1e-5
SCALE = 1.0 / 8.0


def _emit(nc, tc, io):
    from contextlib import ExitStack

    with ExitStack() as ctx:
        const = ctx.enter_context(tc.tile_pool(name="const", bufs=1))
        xchain = ctx.enter_context(tc.tile_pool(name="xchain", bufs=2))
        smse = ctx.enter_context(tc.tile_pool(name="smse", bufs=2))
        smln = ctx.enter_context(tc.tile_pool(name="smln", bufs=1))
        lnp = ctx.enter_context(tc.tile_pool(name="lnp", bufs=1))
        psum = ctx.enter_context(tc.tile_pool(name="psum", bufs=2, space="PSUM"))

        maskT = const.tile([128, 128], BF16, tag="maskT")
        nc.sync.dma_start(maskT[:], io["maskT"][:])
        biases = const.tile([128, 5, 7], F32, tag="biases")
        nc.sync.dma_start(biases[:], io["bias_pack"][:])
        lq_bp = lambda m: biases[:, m, 0:1]
        lk_bp = lambda m: biases[:, m, 1:2]
        lo_bp = lambda m: biases[:, m, 2:3]
        gq_bp = lambda m: biases[:, m, 3:4]
        gk_bp = lambda m: biases[:, m, 4:5]
        go_bp = lambda m: biases[:, m, 5:6]
        f2_bp = lambda m: biases[:, m, 6:7]
        f1_bp = const.tile([128, HCH], F32, tag="f1_bp")
        nc.sync.dma_start(f1_bp[:], io["f1_bp"][:])
        vb65 = const.tile([65, 2, H], F32, tag="vb65")
        nc.sync.dma_start(vb65[:], io["vb65"][:])
        ones65 = const.tile([65, 64], F32, tag="ones65")
        nc.vector.memset(ones65[:], 1.0)
        ones_p0 = const.tile([1, 128], F32, tag="ones_p0")
        nc.vector.memset(ones_p0[:], 1.0)
        ones128b = const.tile([128, 1], BF16, tag="ones128b")
        nc.vector.memset(ones128b[:], 1.0)
        eps_t = const.tile([1, 1], F32, tag="eps")
        nc.vector.memset(eps_t[:], EPS)

        def make_loader(pool):
            def load_w(name, shape, dt_):
                t = pool.tile(shape, dt_, tag="w640", name="wt")
                nc.sync.dma_start(t[:], io[name][:])
                return t
            return load_w

        def proj_fm(x_sb, w_sb, out_sb, bias_fn):
            for sc in range(2):
                for m in range(DCH):
                    pt = psum.tile([128, 512], F32, tag="pp")
                    for k in range(DCH):
                        nc.tensor.matmul(
                            pt[:],
                            w_sb[:, k, m * 128:(m + 1) * 128],
                            x_sb[:, k, sc * 512:(sc + 1) * 512],
                            start=(k == 0),
                            stop=(k == DCH - 1),
                        )
                    nc.vector.tensor_scalar_add(
                        out_sb[:, m, sc * 512:(sc + 1) * 512], pt[:], bias_fn(m)
                    )

        def ln_to(x_in, x_out):
            for sc in range(2):
                sl = slice(sc * 512, (sc + 1) * 512)
                xb = lnp.tile([128, DCH, 512], BF16, tag="ln_xb")
                xq = lnp.tile([128, DCH, 512], BF16, tag="ln_xq")
                for k in range(DCH):
                    nc.vector.tensor_copy(xb[:, k, :], x_in[:, k, sl])
                    nc.vector.tensor_tensor(
                        xq[:, k, :], x_in[:, k, sl], x_in[:, k, sl], OP.mult
                    )
                pm = psum.tile([1, 512], F32, tag="pp")
                pq = psum.tile([1, 512], F32, tag="pp")
                for k in range(DCH):
                    nc.tensor.matmul(pm[:], ones128b[:], xb[:, k, :],
                                     start=(k == 0), stop=(k == DCH - 1))
                for k in range(DCH):
                    nc.tensor.matmul(pq[:], ones128b[:], xq[:, k, :],
                                     start=(k == 0), stop=(k == DCH - 1))
                mean = smln.tile([1, 512], F32, tag="ln_mean")
                var = smln.tile([1, 512], F32, tag="ln_var")
                rstd = smln.tile([1, 512], F32, tag="ln_rstd")
                cc = smln.tile([1, 512], F32, tag="ln_c")
                nc.vector.tensor_scalar_mul(mean[:], pm[:], 1.0 / D)
                nc.vector.tensor_scalar_mul(var[:], pq[:], 1.0 / D)
                nc.vector.tensor_tensor(cc[:], mean[:], mean[:], OP.mult)
                nc.vector.tensor_tensor(var[:], var[:], cc[:], OP.subtract)
                nc.scalar.activation(rstd[:], var[:], AF.Sqrt, bias=eps_t[:])
                nc.vector.reciprocal(rstd[:], rstd[:])
                nc.vector.tensor_tensor(cc[:], mean[:], rstd[:], OP.mult)
                nc.vector.tensor_scalar_mul(cc[:], cc[:], -1.0)
                pa = psum.tile([128, 512], F32, tag="pp")
                pc = psum.tile([128, 512], F32, tag="pp")
                nc.tensor.matmul(pa[:], ones_p0[:], rstd[:],
                                 start=True, stop=True)
                nc.tensor.matmul(pc[:], ones_p0[:], cc[:],
                                 start=True, stop=True)
                for k in range(DCH):
                    nc.vector.tensor_tensor(x_out[:, k, sl], x_in[:, k, sl],
                                            pa[:], OP.mult)
                    nc.vector.tensor_tensor(x_out[:, k, sl], x_out[:, k, sl],
                                            pc[:], OP.add)

        def attn_finish(pacc, pj, sc, vb_base, pool, ow_sb, xr_sb):
            sl = slice(sc * 512, (sc + 1) * 512)
            ats = []
            for i in range(2):
                ps_a = pacc[i]
                se = smse.tile([65, 512], F32, tag="sumexp")
                nc.vector.tensor_copy(se[64:65, :], ps_a[64:65, :])
                nc.vector.reciprocal(se[64:65, :], se[64:65, :])
                pr = psum.tile([64, 512], F32, tag="pp")
                nc.tensor.matmul(pr[:], ones65[64:65, :],
                                 se[64:65, :], start=True, stop=True)
                prs = smse.tile([64, 512], F32, tag="prs")
                nc.vector.tensor_copy(prs[:], pr[:])
                at = smse.tile([64, 512], BF16, tag=f"attn{i}", name=f"at{i}")
                nc.vector.tensor_tensor(at[:], ps_a[0:64, :], prs[:], OP.mult)
                nc.vector.tensor_scalar_add(
                    at[:], at[:], vb_base[0:64, 2 * pj + i:2 * pj + i + 1])
                ats.append(at)
            for m in range(DCH):
                po = psum.tile([128, 512], F32, tag="pp")
                for i in range(2):
                    h = 2 * pj + i
                    nc.tensor.matmul(po[:],
                                     ow_sb[:, h, m * 128:(m + 1) * 128],
                                     ats[i][:],
                                     start=(i == 0), stop=(i == 1))
                nc.vector.tensor_tensor(xr_sb[:, m, sl], po[:],
                                        xr_sb[:, m, sl], OP.add)

        x_sb = xchain.tile([128, DCH, T], F32, tag="x")
        nc.sync.dma_start(x_sb[:], io["xT"][:])

        with tc.tile_pool(name="pL", bufs=1) as pL, \
             tc.tile_pool(name="pLw", bufs=2) as pLw, \
             tc.tile_pool(name="pLe", bufs=6) as pLe, \
             tc.tile_pool(name="psumL", bufs=2, space="PSUM") as psumL, \
             tc.tile_pool(name="psaccL", bufs=1, space="PSUM") as psaccL:
            load_w = make_loader(pLw)
            q_sb = pL.tile([128, DCH, T], BF16, tag="q")
            k_sb = pL.tile([128, DCH, T], BF16, tag="k")
            v_sb = pL.tile([128, NW, H, 65], BF16, tag="v")
            xb16 = pL.tile([128, DCH, T], BF16, tag="xb16")
            for k in range(DCH):
                nc.vector.tensor_copy(xb16[:, k, :], x_sb[:, k, :])

            lw = load_w("lq_w", [128, DCH, D], BF16)
            proj_fm(xb16, lw, q_sb, lq_bp)
            lw = load_w("lk_w", [128, DCH, D], BF16)
            proj_fm(xb16, lw, k_sb, lk_bp)

            nc.vector.memset(v_sb[:, :, :, 64:65], 1.0)
            lw = load_w("lv_w", [128, DCH, D], BF16)
            for st in range(NW):
                ssl = slice(st * 128, (st + 1) * 128)
                for j in range(2):
                    pt = psum.tile([128, 320], F32, tag="pp")
                    for k in range(DCH):
                        nc.tensor.matmul(
                            pt[:],
                            xb16[:, k, ssl],
                            lw[:, k, j * 320:(j + 1) * 320],
                            start=(k == 0),
                            stop=(k == DCH - 1),
                        )
                    for hh in range(5):
                        h = j * 5 + hh
                        nc.vector.tensor_copy(
                            v_sb[:, st, h, 0:64], pt[:, hh * 64:(hh + 1) * 64]
                        )

            for m in range(DCH):
                nc.vector.tensor_scalar_add(x_sb[:, m, :], x_sb[:, m, :],
                                            lo_bp(m))
            low = load_w("lo_w", [64, H, D], BF16)

            for pj in range(DCH):
                for sc in range(2):
                    pacc = [
                        psaccL.tile([65, 512], F32, tag=f"pacc{i}",
                                    name=f"pacc{i}") for i in range(2)
                    ]
                    for i in range(2):
                        h = 2 * pj + i
                        hp = slice(i * 64, (i + 1) * 64)
                        ps_s = psumL.tile([128, 4, 128], F32, tag=f"psl{i}",
                                          name=f"psl{i}")
                        for wi in range(4):
                            w = sc * 4 + wi
                            wsl = slice(w * 128, (w + 1) * 128)
                            nc.tensor.matmul(ps_s[:, wi, :], k_sb[hp, pj, wsl],
                                             q_sb[hp, pj, wsl],
                                             start=True, stop=True)
                        ex = pLe.tile([128, 4, 128], BF16, tag=f"expl{i}",
                                      name=f"ex{i}")
                        nc.scalar.activation(ex[:], ps_s[:], AF.Exp,
                                             scale=SCALE)
                        nc.vector.tensor_tensor(
                            ex[:], ex[:],
                            maskT[:, None, :].to_broadcast((128, 4, 128)),
                            OP.mult)
                        for wi in range(4):
                            nc.tensor.matmul(
                                pacc[i][:, wi * 128:(wi + 1) * 128],
                                v_sb[:, sc * 4 + wi, h, :], ex[:, wi, :],
                                start=True, stop=True,
                            )
                    attn_finish(pacc, pj, sc, vb65[:, 0, :], pLe, low, x_sb)

        x1 = xchain.tile([128, DCH, T], F32, tag="x")
        ln_to(x_sb, x1)

        with tc.tile_pool(name="pG", bufs=1) as pG, \
             tc.tile_pool(name="pGw", bufs=1) as pGw, \
             tc.tile_pool(name="pGb", bufs=2) as pGb, \
             tc.tile_pool(name="pGe", bufs=3) as pGe, \
             tc.tile_pool(name="psumG", bufs=1, space="PSUM") as psumG, \
             tc.tile_pool(name="psaccG", bufs=2, space="PSUM") as psaccG:
            load_w = make_loader(pGw)
            q2 = pG.tile([128, DCH, T], BF16, tag="q2")
            kF = pG.tile([128, DCH, NB], BF16, tag="kF")
            vG = pG.tile([128, NBC, H, 65], BF16, tag="vG")

            x1b = pG.tile([128, DCH, T], BF16, tag="x1b")
            for k in range(DCH):
                nc.vector.tensor_copy(x1b[:, k, :], x1[:, k, :])
            gw = load_w("gq_w", [128, DCH, D], BF16)
            proj_fm(x1b, gw, q2, gq_bp)

            nc.vector.memset(vG[:, :, :, 64:65], 1.0)
            gkv = load_w("gkv_w", [128, DCH, 1280], BF16)
            for ng in range(8):
                nsl = slice(ng * 512, (ng + 1) * 512)
                bm = pGb.tile([128, DCH, 512], BF16, tag="bm")
                nc.sync.dma_start(bm[:], io["bmT"][:, :, nsl])
                for m in range(DCH):
                    pt = psum.tile([128, 512], F32, tag="pp")
                    for k in range(DCH):
                        nc.tensor.matmul(pt[:], gkv[:, k, m * 128:(m + 1) * 128],
                                         bm[:, k, :],
                                         start=(k == 0), stop=(k == DCH - 1))
                    nc.vector.tensor_scalar_add(kF[:, m, nsl], pt[:], gk_bp(m))
                for nb in range(4):
                    n = ng * 4 + nb
                    bsl = slice(nb * 128, (nb + 1) * 128)
                    for j in range(2):
                        pt = psum.tile([128, 320], F32, tag="pp")
                        for k in range(DCH):
                            nc.tensor.matmul(
                                pt[:], bm[:, k, bsl],
                                gkv[:, k, 640 + j * 320:640 + (j + 1) * 320],
                                start=(k == 0), stop=(k == DCH - 1))
                        for hh in range(5):
                            h = j * 5 + hh
                            nc.vector.tensor_copy(
                                vG[:, n, h, 0:64], pt[:, hh * 64:(hh + 1) * 64]
                            )

            for m in range(DCH):
                nc.vector.tensor_scalar_add(x1[:, m, :], x1[:, m, :], go_bp(m))
            gow = load_w("go_w", [64, H, D], BF16)

            for pj in range(DCH):
                pacc = [
                    [psaccG.tile([65, 512], F32, tag=f"pacc{i}",
                                 name=f"pacc{i}{sc}") for sc in range(2)]
                    for i in range(2)
                ]
                for n in range(NBC):
                    nsl = slice(n * 128, (n + 1) * 128)
                    for i in range(2):
                        h = 2 * pj + i
                        hp = slice(i * 64, (i + 1) * 64)
                        ps_s = psumG.tile([128, 1024], F32, tag="psg",
                                          name="psg")
                        for sc in range(2):
                            nc.tensor.matmul(
                                ps_s[:, sc * 512:(sc + 1) * 512],
                                kF[hp, pj, nsl],
                                q2[hp, pj, sc * 512:(sc + 1) * 512],
                                start=True, stop=True)
                        ex = pGe.tile([128, 1024], BF16, tag=f"expg{i}",
                                      name=f"ex{i}")
                        nc.scalar.activation(ex[:], ps_s[:], AF.Exp,
                                             scale=SCALE)
                        for sc in range(2):
                            nc.tensor.matmul(
                                pacc[i][sc][:], vG[:, n, h, :],
                                ex[:, sc * 512:(sc + 1) * 512],
                                start=(n == 0), stop=(n == NBC - 1))
                for sc in range(2):
                    attn_finish([pacc[0][sc], pacc[1][sc]], pj, sc,
                                vb65[:, 1, :], pGe, gow, x1)

        x2 = xchain.tile([128, DCH, T], F32, tag="x")
        ln_to(x1, x2)

        with tc.tile_pool(name="pF", bufs=1) as pF:
            f1w = pF.tile([128, DCH, HID], BF16, tag="f1w")
            nc.sync.dma_start(f1w[:], io["f1_w"][:])
            f2w = pF.tile([128, HCH, D], BF16, tag="f2w")
            nc.sync.dma_start(f2w[:], io["f2_w"][:])
            x2b = pF.tile([128, DCH, T], BF16, tag="x2b")
            for k in range(DCH):
                nc.vector.tensor_copy(x2b[:, k, :], x2[:, k, :])
            xr3 = x2
            for m in range(DCH):
                nc.vector.tensor_scalar_add(xr3[:, m, :], xr3[:, m, :], f2_bp(m))
            for sc in range(2):
                sl = slice(sc * 512, (sc + 1) * 512)
                hsb = pF.tile([128, HCH, 512], BF16, tag=f"h{sc}")
                for mh in range(HCH):
                    pt = psum.tile([128, 512], F32, tag="pp")
                    for k in range(DCH):
                        nc.tensor.matmul(pt[:],
                                         f1w[:, k, mh * 128:(mh + 1) * 128],
                                         x2b[:, k, sl],
                                         start=(k == 0), stop=(k == DCH - 1))
                    nc.scalar.activation(hsb[:, mh, :], pt[:], AF.Gelu,
                                         bias=f1_bp[:, mh:mh + 1])
                for m in range(DCH):
                    pt = psum.tile([128, 512], F32, tag="pp")
                    for k in range(HCH):
                        nc.tensor.matmul(pt[:],
                                         f2w[:, k, m * 128:(m + 1) * 128],
                                         hsb[:, k, :],
                                         start=(k == 0), stop=(k == HCH - 1))
                    nc.vector.tensor_tensor(xr3[:, m, sl], pt[:], xr3[:, m, sl],
                                            OP.add)

        out_sb = xchain.tile([128, DCH, T], F32, tag="x")
        ln_to(xr3, out_sb)
        nc.sync.dma_start(io["out"][:], out_sb[:])



_CACHE = {}


def _build():
    if "nc" in _CACHE:
        return _CACHE["nc"]
    nc = bacc.Bacc("TRN2", target_bir_lowering=False, debug=False)

    specs = {
        "xT": ([128, DCH, T], F32),
        "bmT": ([128, DCH, NB], BF16),
        "lq_w": ([128, DCH, D], BF16),
        "lk_w": ([128, DCH, D], BF16),
        "lv_w": ([128, DCH, D], BF16),
        "lo_w": ([64, H, D], BF16),
        "gq_w": ([128, DCH, D], BF16),
        "gkv_w": ([128, DCH, 1280], BF16),
        "go_w": ([64, H, D], BF16),
        "f1_w": ([128, DCH, HID], BF16),
        "f2_w": ([128, HCH, D], BF16),
        "bias_pack": ([128, 5, 7], F32),
        "f1_bp": ([128, HCH], F32),
        "vb65": ([65, 2, H], F32),
        "maskT": ([128, 128], BF16),
    }
    io = {}
    for name, (shape, dt_) in specs.items():
        io[name] = nc.dram_tensor(name, shape, dt_, kind="ExternalInput").ap()
    io["out"] = nc.dram_tensor("out", [128, DCH, T], F32,
                               kind="ExternalOutput").ap()

    with tile.TileContext(nc) as tc:
        _emit(nc, tc, io)
    nc.compile()
    _CACHE["nc"] = nc
    return nc


def _fm(a):
    Tn, Dn = a.shape
    return np.ascontiguousarray(
        a.T.reshape(Dn // 128, 128, Tn).transpose(1, 0, 2))


def _wfm(w):
    din, dout = w.shape
    return np.ascontiguousarray(
        w.reshape(din // 128, 128, dout).transpose(1, 0, 2))


def _whead(w):
    return np.ascontiguousarray(w.reshape(H, DH, -1).transpose(1, 0, 2))


def _bp(b):
    return np.ascontiguousarray(b.reshape(-1, 128).T)


def prepare_in_maps(inputs):
    f32 = lambda a: np.asarray(a, dtype=np.float32)

    x = f32(inputs["x"]).reshape(2 * 4096, D)
    bias_pack = np.ascontiguousarray(np.stack(
        [_bp(f32(inputs["lq_b"])), _bp(f32(inputs["lk_b"])),
         _bp(f32(inputs["lo_b"])), _bp(f32(inputs["gq_b"])),
         _bp(f32(inputs["gkv_b"])[:D]), _bp(f32(inputs["go_b"])),
         _bp(f32(inputs["f2_b"]))],
        axis=2))

    vb65 = np.zeros((65, 2, H), np.float32)
    vb65[:64, 0, :] = f32(inputs["lv_b"]).reshape(H, DH).T
    vb65[:64, 1, :] = f32(inputs["gkv_b"])[D:].reshape(H, DH).T

    maskT = (np.arange(128)[:, None] <= np.arange(128)[None, :]).astype(BF)

    shared = {
        "bmT": _fm(f32(inputs["bucket_matrix"])).astype(BF),
        "lq_w": _wfm(f32(inputs["lq_w"])).astype(BF),
        "lk_w": _wfm(f32(inputs["lk_w"])).astype(BF),
        "lv_w": _wfm(f32(inputs["lv_w"])).astype(BF),
        "lo_w": _whead(f32(inputs["lo_w"])).astype(BF),
        "gq_w": _wfm(f32(inputs["gq_w"])).astype(BF),
        "gkv_w": _wfm(f32(inputs["gkv_w"])).astype(BF),
        "go_w": _whead(f32(inputs["go_w"])).astype(BF),
        "f1_w": _wfm(f32(inputs["f1_w"])).astype(BF),
        "f2_w": _wfm(f32(inputs["f2_w"])).astype(BF),
        "bias_pack": bias_pack,
        "f1_bp": _bp(f32(inputs["f1_b"])),
        "vb65": vb65,
        "maskT": maskT,
    }
    in_maps = []
    for c in range(N_CORES):
        m = dict(shared)
        m["xT"] = _fm(x[c * T:(c + 1) * T])
        in_maps.append(m)
    return in_maps


def assemble(results):
    out = np.empty((2 * 4096, D), np.float32)
    for c in range(N_CORES):
        o = np.asarray(results[c]["out"])
        out[c * T:(c + 1) * T] = o.transpose(2, 1, 0).reshape(T, D)
    return out.reshape(2, 4096, D)


def kernel(**inputs):
    nc = _build()
    in_maps = prepare_in_maps(inputs)
    res = run_bass_kernel_spmd(nc, in_maps, list(range(N_CORES)))
    return assemble(res.results)


# revision 17
# speedup vs baseline: 1.3336x; 1.3336x over previous
import sys

sys.path.insert(0, "/opt/trn_rl_repo")

import numpy as np
import ml_dtypes

import concourse.bass as bass
import concourse.mybir as mybir
import concourse.tile as tile
from concourse import bacc
from concourse.bass_utils import run_bass_kernel_spmd

F32 = mybir.dt.float32
BF16 = mybir.dt.bfloat16
AF = mybir.ActivationFunctionType
OP = mybir.AluOpType
BF = ml_dtypes.bfloat16

D = 640
DCH = 5
H = 10
DH = 64
T = 1024
W = 128
NW = T // W
NB = 4096
NBC = NB // 128
HID = 4 * D
HCH = HID // 128
N_CORES = 8
EPS = 1e-5
SCALE = 1.0 / 8.0


def _emit(nc, tc, io):
    from contextlib import ExitStack

    with ExitStack() as ctx:
        const = ctx.enter_context(tc.tile_pool(name="const", bufs=1))
        xchain = ctx.enter_context(tc.tile_pool(name="xchain", bufs=2))
        smse = ctx.enter_context(tc.tile_pool(name="smse", bufs=2))
        smln = ctx.enter_context(tc.tile_pool(name="smln", bufs=1))
        lnp = ctx.enter_context(tc.tile_pool(name="lnp", bufs=1))
        psum = ctx.enter_context(tc.tile_pool(name="psum", bufs=2, space="PSUM"))

        maskT = const.tile([128, 128], BF16, tag="maskT")
        nc.sync.dma_start(maskT[:], io["maskT"][:])
        biases = const.tile([128, 5, 7], F32, tag="biases")
        nc.sync.dma_start(biases[:], io["bias_pack"][:])
        lq_bp = lambda m: biases[:, m, 0:1]
        lk_bp = lambda m: biases[:, m, 1:2]
        lo_bp = lambda m: biases[:, m, 2:3]
        gq_bp = lambda m: biases[:, m, 3:4]
        gk_bp = lambda m: biases[:, m, 4:5]
        go_bp = lambda m: biases[:, m, 5:6]
        f2_bp = lambda m: biases[:, m, 6:7]
        f1_bp = const.tile([128, HCH], F32, tag="f1_bp")
        nc.sync.dma_start(f1_bp[:], io["f1_bp"][:])
        vb65 = const.tile([65, 2, H], F32, tag="vb65")
        nc.sync.dma_start(vb65[:], io["vb65"][:])
        ones65 = const.tile([65, 64], F32, tag="ones65")
        nc.vector.memset(ones65[:], 1.0)
        ones_p0 = const.tile([1, 128], F32, tag="ones_p0")
        nc.vector.memset(ones_p0[:], 1.0)
        ones128b = const.tile([128, 1], BF16, tag="ones128b")
        nc.vector.memset(ones128b[:], 1.0)
        eps_t = const.tile([1, 1], F32, tag="eps")
        nc.vector.memset(eps_t[:], EPS)

        def make_loader(pool):
            def load_w(name, shape, dt_):
                t = pool.tile(shape, dt_, tag="w640", name="wt")
                nc.sync.dma_start(t[:], io[name][:])
                return t
            return load_w

        def proj_fm(x_sb, w_sb, out_sb, bias_fn):
            for sc in range(2):
                for m in range(DCH):
                    pt = psum.tile([128, 512], F32, tag="pp")
                    for k in range(DCH):
                        nc.tensor.matmul(
                            pt[:],
                            w_sb[:, k, m * 128:(m + 1) * 128],
                            x_sb[:, k, sc * 512:(sc + 1) * 512],
                            start=(k == 0),
                            stop=(k == DCH - 1),
                        )
                    nc.vector.tensor_scalar_add(
                        out_sb[:, m, sc * 512:(sc + 1) * 512], pt[:], bias_fn(m)
                    )

        def ln_to(x_in, x_out):
            for sc in range(2):
                sl = slice(sc * 512, (sc + 1) * 512)
                xb = lnp.tile([128, DCH, 512], BF16, tag="ln_xb")
                xq = lnp.tile([128, DCH, 512], BF16, tag="ln_xq")
                for k in range(DCH):
                    nc.vector.tensor_copy(xb[:, k, :], x_in[:, k, sl])
                    nc.vector.tensor_tensor(
                        xq[:, k, :], x_in[:, k, sl], x_in[:, k, sl], OP.mult
                    )
                pm = psum.tile([1, 512], F32, tag="pp")
                pq = psum.tile([1, 512], F32, tag="pp")
                for k in range(DCH):
                    nc.tensor.matmul(pm[:], ones128b[:], xb[:, k, :],
                                     start=(k == 0), stop=(k == DCH - 1))
                for k in range(DCH):
                    nc.tensor.matmul(pq[:], ones128b[:], xq[:, k, :],
                                     start=(k == 0), stop=(k == DCH - 1))
                mean = smln.tile([1, 512], F32, tag="ln_mean")
                var = smln.tile([1, 512], F32, tag="ln_var")
                rstd = smln.tile([1, 512], F32, tag="ln_rstd")
                cc = smln.tile([1, 512], F32, tag="ln_c")
                nc.vector.tensor_scalar_mul(mean[:], pm[:], 1.0 / D)
                nc.vector.tensor_scalar_mul(var[:], pq[:], 1.0 / D)
                nc.vector.tensor_tensor(cc[:], mean[:], mean[:], OP.mult)
                nc.vector.tensor_tensor(var[:], var[:], cc[:], OP.subtract)
                nc.scalar.activation(rstd[:], var[:], AF.Sqrt, bias=eps_t[:])
                nc.vector.reciprocal(rstd[:], rstd[:])
                nc.vector.tensor_tensor(cc[:], mean[:], rstd[:], OP.mult)
                nc.vector.tensor_scalar_mul(cc[:], cc[:], -1.0)
                pa = psum.tile([128, 512], F32, tag="pp")
                pc = psum.tile([128, 512], F32, tag="pp")
                nc.tensor.matmul(pa[:], ones_p0[:], rstd[:],
                                 start=True, stop=True)
                nc.tensor.matmul(pc[:], ones_p0[:], cc[:],
                                 start=True, stop=True)
                for k in range(DCH):
                    nc.vector.tensor_tensor(x_out[:, k, sl], x_in[:, k, sl],
                                            pa[:], OP.mult)
                    nc.vector.tensor_tensor(x_out[:, k, sl], x_out[:, k, sl],
                                            pc[:], OP.add)

        def attn_finish(pacc, pj, sc, vb_base, pool, ow_sb, xr_sb):
            sl = slice(sc * 512, (sc + 1) * 512)
            ats = []
            for i in range(2):
                ps_a = pacc[i]
                se = smse.tile([65, 512], F32, tag="sumexp")
                nc.vector.tensor_copy(se[64:65, :], ps_a[64:65, :])
                nc.vector.reciprocal(se[64:65, :], se[64:65, :])
                pr = psum.tile([64, 512], F32, tag="pp")
                nc.tensor.matmul(pr[:], ones65[64:65, :],
                                 se[64:65, :], start=True, stop=True)
                prs = smse.tile([64, 512], F32, tag="prs")
                nc.vector.tensor_copy(prs[:], pr[:])
                at = smse.tile([64, 512], BF16, tag=f"attn{i}", name=f"at{i}")
                nc.vector.tensor_tensor(at[:], ps_a[0:64, :], prs[:], OP.mult)
                nc.vector.tensor_scalar_add(
                    at[:], at[:], vb_base[0:64, 2 * pj + i:2 * pj + i + 1])
                ats.append(at)
            for m in range(DCH):
                po = psum.tile([128, 512], F32, tag="pp")
                for i in range(2):
                    h = 2 * pj + i
                    nc.tensor.matmul(po[:],
                                     ow_sb[:, h, m * 128:(m + 1) * 128],
                                     ats[i][:],
                                     start=(i == 0), stop=(i == 1))
                nc.vector.tensor_tensor(xr_sb[:, m, sl], po[:],
                                        xr_sb[:, m, sl], OP.add)

        x_sb = xchain.tile([128, DCH, T], F32, tag="x")
        nc.sync.dma_start(x_sb[:], io["xT"][:])

        with tc.tile_pool(name="pL", bufs=1) as pL, \
             tc.tile_pool(name="pLw", bufs=2) as pLw, \
             tc.tile_pool(name="pLe", bufs=6) as pLe, \
             tc.tile_pool(name="psumL", bufs=2, space="PSUM") as psumL, \
             tc.tile_pool(name="psaccL", bufs=1, space="PSUM") as psaccL:
            load_w = make_loader(pLw)
            q_sb = pL.tile([128, DCH, T], BF16, tag="q")
            k_sb = pL.tile([128, DCH, T], BF16, tag="k")
            v_sb = pL.tile([128, NW, H, 65], BF16, tag="v")
            xb16 = pL.tile([128, DCH, T], BF16, tag="xb16")
            for k in range(DCH):
                nc.vector.tensor_copy(xb16[:, k, :], x_sb[:, k, :])

            lw = load_w("lq_w", [128, DCH, D], BF16)
            proj_fm(xb16, lw, q_sb, lq_bp)
            lw = load_w("lk_w", [128, DCH, D], BF16)
            proj_fm(xb16, lw, k_sb, lk_bp)

            nc.vector.memset(v_sb[:, :, :, 64:65], 1.0)
            lw = load_w("lv_w", [128, DCH, D], BF16)
            for st in range(NW):
                ssl = slice(st * 128, (st + 1) * 128)
                for j in range(2):
                    pt = psum.tile([128, 320], F32, tag="pp")
                    for k in range(DCH):
                        nc.tensor.matmul(
                            pt[:],
                            xb16[:, k, ssl],
                            lw[:, k, j * 320:(j + 1) * 320],
                            start=(k == 0),
                            stop=(k == DCH - 1),
                        )
                    for hh in range(5):
                        h = j * 5 + hh
                        nc.vector.tensor_copy(
                            v_sb[:, st, h, 0:64], pt[:, hh * 64:(hh + 1) * 64]
                        )

            for m in range(DCH):
                nc.vector.tensor_scalar_add(x_sb[:, m, :], x_sb[:, m, :],
                                            lo_bp(m))
            low = load_w("lo_w", [64, H, D], BF16)

            for pj in range(DCH):
                for sc in range(2):
                    pacc = [
                        psaccL.tile([65, 512], F32, tag=f"pacc{i}",
                                    name=f"pacc{i}") for i in range(2)
                    ]
                    for i in range(2):
                        h = 2 * pj + i
                        hp = slice(i * 64, (i + 1) * 64)
                        ps_s = psumL.tile([128, 4, 128], F32, tag=f"psl{i}",
                                          name=f"psl{i}")
                        for wi in range(4):
                            w = sc * 4 + wi
                            wsl = slice(w * 128, (w + 1) * 128)
                            nc.tensor.matmul(ps_s[:, wi, :], k_sb[hp, pj, wsl],
                                             q_sb[hp, pj, wsl],
                                             start=True, stop=True)
                        ex = pLe.tile([128, 4, 128], BF16, tag=f"expl{i}",
                                      name=f"ex{i}")
                        nc.scalar.activation(ex[:], ps_s[:], AF.Exp,
                                             scale=SCALE)
                        nc.vector.tensor_tensor(
                            ex[:], ex[:],
                            maskT[:, None, :].to_broadcast((128, 4, 128)),
                            OP.mult)
                        for wi in range(4):
                            nc.tensor.matmul(
                                pacc[i][:, wi * 128:(wi + 1) * 128],
                                v_sb[:, sc * 4 + wi, h, :], ex[:, wi, :],
                                start=True, stop=True,
                            )
                    attn_finish(pacc, pj, sc, vb65[:, 0, :], pLe, low, x_sb)

        x1 = xchain.tile([128, DCH, T], F32, tag="x")
        ln_to(x_sb, x1)

        with tc.tile_pool(name="pG", bufs=1) as pG, \
             tc.tile_pool(name="pGw", bufs=1) as pGw, \
             tc.tile_pool(name="pGb", bufs=2) as pGb, \
             tc.tile_pool(name="pGe", bufs=4) as pGe, \
             tc.tile_pool(name="psumG", bufs=2, space="PSUM") as psumG, \
             tc.tile_pool(name="psaccG", bufs=1, space="PSUM") as psaccG:
            load_w = make_loader(pGw)
            q2 = pG.tile([128, DCH, T], BF16, tag="q2")
            kF = pG.tile([128, DCH, NB], BF16, tag="kF")
            vG = pG.tile([128, NBC, H, 65], BF16, tag="vG")

            x1b = pG.tile([128, DCH, T], BF16, tag="x1b")
            for k in range(DCH):
                nc.vector.tensor_copy(x1b[:, k, :], x1[:, k, :])
            gw = load_w("gq_w", [128, DCH, D], BF16)
            proj_fm(x1b, gw, q2, gq_bp)

            nc.vector.memset(vG[:, :, :, 64:65], 1.0)
            gkv = load_w("gkv_w", [128, DCH, 1280], BF16)
            for ng in range(8):
                nsl = slice(ng * 512, (ng + 1) * 512)
                bm = pGb.tile([128, DCH, 512], BF16, tag="bm")
                nc.sync.dma_start(bm[:], io["bmT"][:, :, nsl])
                for m in range(DCH):
                    pt = psum.tile([128, 512], F32, tag="pp")
                    for k in range(DCH):
                        nc.tensor.matmul(pt[:], gkv[:, k, m * 128:(m + 1) * 128],
                                         bm[:, k, :],
                                         start=(k == 0), stop=(k == DCH - 1))
                    nc.vector.tensor_scalar_add(kF[:, m, nsl], pt[:], gk_bp(m))
                for nb in range(4):
                    n = ng * 4 + nb
                    bsl = slice(nb * 128, (nb + 1) * 128)
                    for j in range(2):
                        pt = psum.tile([128, 320], F32, tag="pp")
                        for k in range(DCH):
                            nc.tensor.matmul(
                                pt[:], bm[:, k, bsl],
                                gkv[:, k, 640 + j * 320:640 + (j + 1) * 320],
                                start=(k == 0), stop=(k == DCH - 1))
                        for hh in range(5):
                            h = j * 5 + hh
                            nc.vector.tensor_copy(
                                vG[:, n, h, 0:64], pt[:, hh * 64:(hh + 1) * 64]
                            )

            for m in range(DCH):
                nc.vector.tensor_scalar_add(x1[:, m, :], x1[:, m, :], go_bp(m))
            gow = load_w("go_w", [64, H, D], BF16)

            for pj in range(DCH):
                for sc in range(2):
                    ssl = slice(sc * 512, (sc + 1) * 512)
                    pacc = [
                        psaccG.tile([65, 512], F32, tag=f"pacc{i}",
                                    name=f"pacc{i}") for i in range(2)
                    ]
                    for n in range(NBC):
                        nsl = slice(n * 128, (n + 1) * 128)
                        for i in range(2):
                            h = 2 * pj + i
                            hp = slice(i * 64, (i + 1) * 64)
                            ps_s = psumG.tile([128, 512], F32, tag=f"psg{i}",
                                              name=f"psg{i}")
                            nc.tensor.matmul(ps_s[:], kF[hp, pj, nsl],
                                             q2[hp, pj, ssl],
                                             start=True, stop=True)
                            ex = pGe.tile([128, 512], BF16, tag=f"expg{i}",
                                          name=f"ex{i}")
                            nc.scalar.activation(ex[:], ps_s[:], AF.Exp,
                                                 scale=SCALE)
                            nc.tensor.matmul(pacc[i][:], vG[:, n, h, :], ex[:],
                                             start=(n == 0),
                                             stop=(n == NBC - 1))
                    attn_finish(pacc, pj, sc, vb65[:, 1, :], pGe, gow, x1)

        x2 = xchain.tile([128, DCH, T], F32, tag="x")
        ln_to(x1, x2)

        with tc.tile_pool(name="pF", bufs=1) as pF:
            f1w = pF.tile([128, DCH, HID], BF16, tag="f1w")
            nc.sync.dma_start(f1w[:], io["f1_w"][:])
            f2w = pF.tile([128, HCH, D], BF16, tag="f2w")
            nc.sync.dma_start(f2w[:], io["f2_w"][:])
            x2b = pF.tile([128, DCH, T], BF16, tag="x2b")
            for k in range(DCH):
                nc.vector.tensor_copy(x2b[:, k, :], x2[:, k, :])
            xr3 = x2
            for m in range(DCH):
                nc.vector.tensor_scalar_add(xr3[:, m, :], xr3[:, m, :], f2_bp(m))
            for sc in range(2):
                sl = slice(sc * 512, (sc + 1) * 512)
                hsb = pF.tile([128, HCH, 512], BF16, tag=f"h{sc}")
                for mh in range(HCH):
                    pt = psum.tile([128, 512], F32, tag="pp")
                    for k in range(DCH):
                        nc.tensor.matmul(pt[:],
                                         f1w[:, k, mh * 128:(mh + 1) * 128],
                                         x2b[:, k, sl],
                                         start=(k == 0), stop=(k == DCH - 1))
                    nc.scalar.activation(hsb[:, mh, :], pt[:], AF.Gelu,
                                         bias=f1_bp[:, mh:mh + 1])
                for m in range(DCH):
                    pt = psum.tile([128, 512], F32, tag="pp")
                    for k in range(HCH):
                        nc.tensor.matmul(pt[:],
                                         f2w[:, k, m * 128:(m + 1) * 128],
                                         hsb[:, k, :],
                                         start=(k == 0), stop=(k == HCH - 1))
                    nc.vector.tensor_tensor(xr3[:, m, sl], pt[:], xr3[:, m, sl],
                                            OP.add)

        out_sb = xchain.tile([128, DCH, T], F32, tag="x")
        ln_to(xr3, out_sb)
        nc.sync.dma_start(io["out"][:], out_sb[:])



_CACHE = {}


def _build():
    if "nc" in _CACHE:
        return _CACHE["nc"]
    nc = bacc.Bacc("TRN2", target_bir_lowering=False, debug=False)

    specs = {
        "xT": ([128, DCH, T], F32),
        "bmT": ([128, DCH, NB], BF16),
        "lq_w": ([128, DCH, D], BF16),
        "lk_w": ([128, DCH, D], BF16),
        "lv_w": ([128, DCH, D], BF16),
        "lo_w": ([64, H, D], BF16),
        "gq_w": ([128, DCH, D], BF16),
        "gkv_w": ([128, DCH, 1280], BF16),
        "go_w": ([64, H, D], BF16),
        "f1_w": ([128, DCH, HID], BF16),
        "f2_w": ([128, HCH, D], BF16),
        "bias_pack": ([128, 5, 7], F32),
        "f1_bp": ([128, HCH], F32),
        "vb65": ([65, 2, H], F32),
        "maskT": ([128, 128], BF16),
    }
    io = {}
    for name, (shape, dt_) in specs.items():
        io[name] = nc.dram_tensor(name, shape, dt_, kind="ExternalInput").ap()
    io["out"] = nc.dram_tensor("out", [128, DCH, T], F32,
                               kind="ExternalOutput").ap()

    with tile.TileContext(nc) as tc:
        _emit(nc, tc, io)
    nc.compile()
    _CACHE["nc"] = nc
    return nc


def _fm(a):
    Tn, Dn = a.shape
    return np.ascontiguousarray(
        a.T.reshape(Dn // 128, 128, Tn).transpose(1, 0, 2))


def _wfm(w):
    din, dout = w.shape
    return np.ascontiguousarray(
        w.reshape(din // 128, 128, dout).transpose(1, 0, 2))


def _whead(w):
    return np.ascontiguousarray(w.reshape(H, DH, -1).transpose(1, 0, 2))


def _bp(b):
    return np.ascontiguousarray(b.reshape(-1, 128).T)


def prepare_in_maps(inputs):
    f32 = lambda a: np.asarray(a, dtype=np.float32)

    x = f32(inputs["x"]).reshape(2 * 4096, D)
    bias_pack = np.ascontiguousarray(np.stack(
        [_bp(f32(inputs["lq_b"])), _bp(f32(inputs["lk_b"])),
         _bp(f32(inputs["lo_b"])), _bp(f32(inputs["gq_b"])),
         _bp(f32(inputs["gkv_b"])[:D]), _bp(f32(inputs["go_b"])),
         _bp(f32(inputs["f2_b"]))],
        axis=2))

    vb65 = np.zeros((65, 2, H), np.float32)
    vb65[:64, 0, :] = f32(inputs["lv_b"]).reshape(H, DH).T
    vb65[:64, 1, :] = f32(inputs["gkv_b"])[D:].reshape(H, DH).T

    maskT = (np.arange(128)[:, None] <= np.arange(128)[None, :]).astype(BF)

    shared = {
        "bmT": _fm(f32(inputs["bucket_matrix"])).astype(BF),
        "lq_w": _wfm(f32(inputs["lq_w"])).astype(BF),
        "lk_w": _wfm(f32(inputs["lk_w"])).astype(BF),
        "lv_w": _wfm(f32(inputs["lv_w"])).astype(BF),
        "lo_w": _whead(f32(inputs["lo_w"])).astype(BF),
        "gq_w": _wfm(f32(inputs["gq_w"])).astype(BF),
        "gkv_w": _wfm(f32(inputs["gkv_w"])).astype(BF),
        "go_w": _whead(f32(inputs["go_w"])).astype(BF),
        "f1_w": _wfm(f32(inputs["f1_w"])).astype(BF),
        "f2_w": _wfm(f32(inputs["f2_w"])).astype(BF),
        "bias_pack": bias_pack,
        "f1_bp": _bp(f32(inputs["f1_b"])),
        "vb65": vb65,
        "maskT": maskT,
    }
    in_maps = []
    for c in range(N_CORES):
        m = dict(shared)
        m["xT"] = _fm(x[c * T:(c + 1) * T])
        in_maps.append(m)
    return in_maps


def assemble(results):
    out = np.empty((2 * 4096, D), np.float32)
    for c in range(N_CORES):
        o = np.asarray(results[c]["out"])
        out[c * T:(c + 1) * T] = o.transpose(2, 1, 0).reshape(T, D)
    return out.reshape(2, 4096, D)


def kernel(**inputs):
    nc = _build()
    in_maps = prepare_in_maps(inputs)
    res = run_bass_kernel_spmd(nc, in_maps, list(range(N_CORES)))
    return assemble(res.results)


# revision 18
# speedup vs baseline: 1.5499x; 1.1622x over previous
import sys

sys.path.insert(0, "/opt/trn_rl_repo")

import numpy as np
import ml_dtypes

import concourse.bass as bass
import concourse.mybir as mybir
import concourse.tile as tile
from concourse import bacc
from concourse.bass_utils import run_bass_kernel_spmd

F32 = mybir.dt.float32
BF16 = mybir.dt.bfloat16
AF = mybir.ActivationFunctionType
OP = mybir.AluOpType
BF = ml_dtypes.bfloat16

D = 640
DCH = 5
H = 10
DH = 64
T = 1024
W = 128
NW = T // W
NB = 4096
NBC = NB // 128
HID = 4 * D
HCH = HID // 128
N_CORES = 8
EPS = 1e-5
SCALE = 1.0 / 8.0


def _emit(nc, tc, io):
    from contextlib import ExitStack

    with ExitStack() as ctx:
        const = ctx.enter_context(tc.tile_pool(name="const", bufs=1))
        xchain = ctx.enter_context(tc.tile_pool(name="xchain", bufs=2))
        smse = ctx.enter_context(tc.tile_pool(name="smse", bufs=2))
        smln = ctx.enter_context(tc.tile_pool(name="smln", bufs=1))
        lnp = ctx.enter_context(tc.tile_pool(name="lnp", bufs=1))
        psum = ctx.enter_context(tc.tile_pool(name="psum", bufs=2, space="PSUM"))

        maskT = const.tile([128, 128], BF16, tag="maskT")
        nc.sync.dma_start(maskT[:], io["maskT"][:])
        biases = const.tile([128, 5, 7], F32, tag="biases")
        nc.sync.dma_start(biases[:], io["bias_pack"][:])
        lq_bp = lambda m: biases[:, m, 0:1]
        lk_bp = lambda m: biases[:, m, 1:2]
        lo_bp = lambda m: biases[:, m, 2:3]
        gq_bp = lambda m: biases[:, m, 3:4]
        gk_bp = lambda m: biases[:, m, 4:5]
        go_bp = lambda m: biases[:, m, 5:6]
        f2_bp = lambda m: biases[:, m, 6:7]
        f1_bp = const.tile([128, HCH], F32, tag="f1_bp")
        nc.sync.dma_start(f1_bp[:], io["f1_bp"][:])
        vb65 = const.tile([65, 2, H], F32, tag="vb65")
        nc.sync.dma_start(vb65[:], io["vb65"][:])
        ones65 = const.tile([65, 64], F32, tag="ones65")
        nc.vector.memset(ones65[:], 1.0)
        ones_p0 = const.tile([1, 128], F32, tag="ones_p0")
        nc.vector.memset(ones_p0[:], 1.0)
        ones128b = const.tile([128, 1], BF16, tag="ones128b")
        nc.vector.memset(ones128b[:], 1.0)
        eps_t = const.tile([1, 1], F32, tag="eps")
        nc.vector.memset(eps_t[:], EPS)

        def make_loader(pool):
            def load_w(name, shape, dt_):
                t = pool.tile(shape, dt_, tag="w640", name="wt")
                nc.sync.dma_start(t[:], io[name][:])
                return t
            return load_w

        def proj_fm(x_sb, w_sb, out_sb, bias_fn):
            for sc in range(2):
                for m in range(DCH):
                    pt = psum.tile([128, 512], F32, tag="pp")
                    for k in range(DCH):
                        nc.tensor.matmul(
                            pt[:],
                            w_sb[:, k, m * 128:(m + 1) * 128],
                            x_sb[:, k, sc * 512:(sc + 1) * 512],
                            start=(k == 0),
                            stop=(k == DCH - 1),
                        )
                    nc.vector.tensor_scalar_add(
                        out_sb[:, m, sc * 512:(sc + 1) * 512], pt[:], bias_fn(m)
                    )

        def ln_to(x_in, x_out):
            for sc in range(2):
                sl = slice(sc * 512, (sc + 1) * 512)
                xb = lnp.tile([128, DCH, 512], BF16, tag="ln_xb")
                xq = lnp.tile([128, DCH, 512], BF16, tag="ln_xq")
                for k in range(DCH):
                    nc.vector.tensor_copy(xb[:, k, :], x_in[:, k, sl])
                    nc.vector.tensor_tensor(
                        xq[:, k, :], x_in[:, k, sl], x_in[:, k, sl], OP.mult
                    )
                pm = psum.tile([1, 512], F32, tag="pp")
                pq = psum.tile([1, 512], F32, tag="pp")
                for k in range(DCH):
                    nc.tensor.matmul(pm[:], ones128b[:], xb[:, k, :],
                                     start=(k == 0), stop=(k == DCH - 1))
                for k in range(DCH):
                    nc.tensor.matmul(pq[:], ones128b[:], xq[:, k, :],
                                     start=(k == 0), stop=(k == DCH - 1))
                mean = smln.tile([1, 512], F32, tag="ln_mean")
                var = smln.tile([1, 512], F32, tag="ln_var")
                rstd = smln.tile([1, 512], F32, tag="ln_rstd")
                cc = smln.tile([1, 512], F32, tag="ln_c")
                nc.vector.tensor_scalar_mul(mean[:], pm[:], 1.0 / D)
                nc.vector.tensor_scalar_mul(var[:], pq[:], 1.0 / D)
                nc.vector.tensor_tensor(cc[:], mean[:], mean[:], OP.mult)
                nc.vector.tensor_tensor(var[:], var[:], cc[:], OP.subtract)
                nc.scalar.activation(rstd[:], var[:], AF.Sqrt, bias=eps_t[:])
                nc.vector.reciprocal(rstd[:], rstd[:])
                nc.vector.tensor_tensor(cc[:], mean[:], rstd[:], OP.mult)
                nc.vector.tensor_scalar_mul(cc[:], cc[:], -1.0)
                pa = psum.tile([128, 512], F32, tag="pp")
                pc = psum.tile([128, 512], F32, tag="pp")
                nc.tensor.matmul(pa[:], ones_p0[:], rstd[:],
                                 start=True, stop=True)
                nc.tensor.matmul(pc[:], ones_p0[:], cc[:],
                                 start=True, stop=True)
                for k in range(DCH):
                    nc.vector.tensor_tensor(x_out[:, k, sl], x_in[:, k, sl],
                                            pa[:], OP.mult)
                    nc.vector.tensor_tensor(x_out[:, k, sl], x_out[:, k, sl],
                                            pc[:], OP.add)

        def attn_finish(pacc, pj, sc, vb_base, pool, ow_sb, xr_sb):
            sl = slice(sc * 512, (sc + 1) * 512)
            ats = []
            for i in range(2):
                ps_a = pacc[i]
                se = smse.tile([65, 512], F32, tag="sumexp")
                nc.vector.tensor_copy(se[64:65, :], ps_a[64:65, :])
                nc.vector.reciprocal(se[64:65, :], se[64:65, :])
                pr = psum.tile([64, 512], F32, tag="pp")
                nc.tensor.matmul(pr[:], ones65[64:65, :],
                                 se[64:65, :], start=True, stop=True)
                prs = smse.tile([64, 512], F32, tag="prs")
                nc.vector.tensor_copy(prs[:], pr[:])
                at = smse.tile([64, 512], BF16, tag=f"attn{i}", name=f"at{i}")
                nc.vector.tensor_tensor(at[:], ps_a[0:64, :], prs[:], OP.mult)
                nc.vector.tensor_scalar_add(
                    at[:], at[:], vb_base[0:64, 2 * pj + i:2 * pj + i + 1])
                ats.append(at)
            for m in range(DCH):
                po = psum.tile([128, 512], F32, tag="pp")
                for i in range(2):
                    h = 2 * pj + i
                    nc.tensor.matmul(po[:],
                                     ow_sb[:, h, m * 128:(m + 1) * 128],
                                     ats[i][:],
                                     start=(i == 0), stop=(i == 1))
                nc.vector.tensor_tensor(xr_sb[:, m, sl], po[:],
                                        xr_sb[:, m, sl], OP.add)

        x_sb = xchain.tile([128, DCH, T], F32, tag="x")
        nc.sync.dma_start(x_sb[:], io["xT"][:])

        with tc.tile_pool(name="pL", bufs=1) as pL, \
             tc.tile_pool(name="pLw", bufs=2) as pLw, \
             tc.tile_pool(name="pLe", bufs=6) as pLe, \
             tc.tile_pool(name="psumL", bufs=2, space="PSUM") as psumL, \
             tc.tile_pool(name="psaccL", bufs=1, space="PSUM") as psaccL:
            load_w = make_loader(pLw)
            q_sb = pL.tile([128, DCH, T], BF16, tag="q")
            k_sb = pL.tile([128, DCH, T], BF16, tag="k")
            v_sb = pL.tile([128, NW, H, 65], BF16, tag="v")
            xb16 = pL.tile([128, DCH, T], BF16, tag="xb16")
            for k in range(DCH):
                nc.vector.tensor_copy(xb16[:, k, :], x_sb[:, k, :])

            lw = load_w("lq_w", [128, DCH, D], BF16)
            proj_fm(xb16, lw, q_sb, lq_bp)
            lw = load_w("lk_w", [128, DCH, D], BF16)
            proj_fm(xb16, lw, k_sb, lk_bp)

            nc.vector.memset(v_sb[:, :, :, 64:65], 1.0)
            lw = load_w("lv_w", [128, DCH, D], BF16)
            for st in range(NW):
                ssl = slice(st * 128, (st + 1) * 128)
                for j in range(2):
                    pt = psum.tile([128, 320], F32, tag="pp")
                    for k in range(DCH):
                        nc.tensor.matmul(
                            pt[:],
                            xb16[:, k, ssl],
                            lw[:, k, j * 320:(j + 1) * 320],
                            start=(k == 0),
                            stop=(k == DCH - 1),
                        )
                    for hh in range(5):
                        h = j * 5 + hh
                        nc.vector.tensor_copy(
                            v_sb[:, st, h, 0:64], pt[:, hh * 64:(hh + 1) * 64]
                        )

            for m in range(DCH):
                nc.vector.tensor_scalar_add(x_sb[:, m, :], x_sb[:, m, :],
                                            lo_bp(m))
            low = load_w("lo_w", [64, H, D], BF16)

            for pj in range(DCH):
                for sc in range(2):
                    pacc = [
                        psaccL.tile([65, 512], F32, tag=f"pacc{i}",
                                    name=f"pacc{i}") for i in range(2)
                    ]
                    ps_s = psumL.tile([128, 2, 4, 128], F32, tag="psl",
                                      name="psl")
                    for i in range(2):
                        hp = slice(i * 64, (i + 1) * 64)
                        for wi in range(4):
                            w = sc * 4 + wi
                            wsl = slice(w * 128, (w + 1) * 128)
                            nc.tensor.matmul(ps_s[:, i, wi, :],
                                             k_sb[hp, pj, wsl],
                                             q_sb[hp, pj, wsl],
                                             start=True, stop=True)
                    ex = pLe.tile([128, 2, 4, 128], BF16, tag="expl",
                                  name="ex")
                    nc.scalar.activation(ex[:], ps_s[:], AF.Exp, scale=SCALE)
                    nc.vector.tensor_tensor(
                        ex[:], ex[:],
                        maskT[:, None, None, :].to_broadcast((128, 2, 4, 128)),
                        OP.mult)
                    for i in range(2):
                        h = 2 * pj + i
                        for wi in range(4):
                            nc.tensor.matmul(
                                pacc[i][:, wi * 128:(wi + 1) * 128],
                                v_sb[:, sc * 4 + wi, h, :], ex[:, i, wi, :],
                                start=True, stop=True,
                            )
                    attn_finish(pacc, pj, sc, vb65[:, 0, :], pLe, low, x_sb)

        x1 = xchain.tile([128, DCH, T], F32, tag="x")
        ln_to(x_sb, x1)

        with tc.tile_pool(name="pG", bufs=1) as pG, \
             tc.tile_pool(name="pGw", bufs=1) as pGw, \
             tc.tile_pool(name="pGb", bufs=2) as pGb, \
             tc.tile_pool(name="pGe", bufs=4) as pGe, \
             tc.tile_pool(name="psumG", bufs=2, space="PSUM") as psumG, \
             tc.tile_pool(name="psaccG", bufs=1, space="PSUM") as psaccG:
            load_w = make_loader(pGw)
            q2 = pG.tile([128, DCH, T], BF16, tag="q2")
            kF = pG.tile([128, DCH, NB], BF16, tag="kF")
            vG = pG.tile([128, NBC, H, 65], BF16, tag="vG")

            x1b = pG.tile([128, DCH, T], BF16, tag="x1b")
            for k in range(DCH):
                nc.vector.tensor_copy(x1b[:, k, :], x1[:, k, :])
            gw = load_w("gq_w", [128, DCH, D], BF16)
            proj_fm(x1b, gw, q2, gq_bp)

            nc.vector.memset(vG[:, :, :, 64:65], 1.0)
            gkv = load_w("gkv_w", [128, DCH, 1280], BF16)
            for ng in range(8):
                nsl = slice(ng * 512, (ng + 1) * 512)
                bm = pGb.tile([128, DCH, 512], BF16, tag="bm")
                nc.sync.dma_start(bm[:], io["bmT"][:, :, nsl])
                for m in range(DCH):
                    pt = psum.tile([128, 512], F32, tag="pp")
                    for k in range(DCH):
                        nc.tensor.matmul(pt[:], gkv[:, k, m * 128:(m + 1) * 128],
                                         bm[:, k, :],
                                         start=(k == 0), stop=(k == DCH - 1))
                    nc.vector.tensor_scalar_add(kF[:, m, nsl], pt[:], gk_bp(m))
                for nb in range(4):
                    n = ng * 4 + nb
                    bsl = slice(nb * 128, (nb + 1) * 128)
                    for j in range(2):
                        pt = psum.tile([128, 320], F32, tag="pp")
                        for k in range(DCH):
                            nc.tensor.matmul(
                                pt[:], bm[:, k, bsl],
                                gkv[:, k, 640 + j * 320:640 + (j + 1) * 320],
                                start=(k == 0), stop=(k == DCH - 1))
                        for hh in range(5):
                            h = j * 5 + hh
                            nc.vector.tensor_copy(
                                vG[:, n, h, 0:64], pt[:, hh * 64:(hh + 1) * 64]
                            )

            for m in range(DCH):
                nc.vector.tensor_scalar_add(x1[:, m, :], x1[:, m, :], go_bp(m))
            gow = load_w("go_w", [64, H, D], BF16)

            for pj in range(DCH):
                for sc in range(2):
                    ssl = slice(sc * 512, (sc + 1) * 512)
                    pacc = [
                        psaccG.tile([65, 512], F32, tag=f"pacc{i}",
                                    name=f"pacc{i}") for i in range(2)
                    ]
                    for n in range(NBC):
                        nsl = slice(n * 128, (n + 1) * 128)
                        ps_s = psumG.tile([128, 2, 512], F32, tag="psg",
                                          name="psg")
                        for i in range(2):
                            hp = slice(i * 64, (i + 1) * 64)
                            nc.tensor.matmul(ps_s[:, i, :], kF[hp, pj, nsl],
                                             q2[hp, pj, ssl],
                                             start=True, stop=True)
                        ex = pGe.tile([128, 2, 512], BF16, tag="expg",
                                      name="ex")
                        nc.scalar.activation(ex[:], ps_s[:], AF.Exp,
                                             scale=SCALE)
                        for i in range(2):
                            h = 2 * pj + i
                            nc.tensor.matmul(pacc[i][:], vG[:, n, h, :],
                                             ex[:, i, :],
                                             start=(n == 0),
                                             stop=(n == NBC - 1))
                    attn_finish(pacc, pj, sc, vb65[:, 1, :], pGe, gow, x1)

        x2 = xchain.tile([128, DCH, T], F32, tag="x")
        ln_to(x1, x2)

        with tc.tile_pool(name="pF", bufs=1) as pF:
            f1w = pF.tile([128, DCH, HID], BF16, tag="f1w")
            nc.sync.dma_start(f1w[:], io["f1_w"][:])
            f2w = pF.tile([128, HCH, D], BF16, tag="f2w")
            nc.sync.dma_start(f2w[:], io["f2_w"][:])
            x2b = pF.tile([128, DCH, T], BF16, tag="x2b")
            for k in range(DCH):
                nc.vector.tensor_copy(x2b[:, k, :], x2[:, k, :])
            xr3 = x2
            for m in range(DCH):
                nc.vector.tensor_scalar_add(xr3[:, m, :], xr3[:, m, :], f2_bp(m))
            for sc in range(2):
                sl = slice(sc * 512, (sc + 1) * 512)
                hsb = pF.tile([128, HCH, 512], BF16, tag=f"h{sc}")
                for mh in range(HCH):
                    pt = psum.tile([128, 512], F32, tag="pp")
                    for k in range(DCH):
                        nc.tensor.matmul(pt[:],
                                         f1w[:, k, mh * 128:(mh + 1) * 128],
                                         x2b[:, k, sl],
                                         start=(k == 0), stop=(k == DCH - 1))
                    nc.scalar.activation(hsb[:, mh, :], pt[:], AF.Gelu,
                                         bias=f1_bp[:, mh:mh + 1])
                for m in range(DCH):
                    pt = psum.tile([128, 512], F32, tag="pp")
                    for k in range(HCH):
                        nc.tensor.matmul(pt[:],
                                         f2w[:, k, m * 128:(m + 1) * 128],
                                         hsb[:, k, :],
                                         start=(k == 0), stop=(k == HCH - 1))
                    nc.vector.tensor_tensor(xr3[:, m, sl], pt[:], xr3[:, m, sl],
                                            OP.add)

        out_sb = xchain.tile([128, DCH, T], F32, tag="x")
        ln_to(xr3, out_sb)
        nc.sync.dma_start(io["out"][:], out_sb[:])



_CACHE = {}


def _build():
    if "nc" in _CACHE:
        return _CACHE["nc"]
    nc = bacc.Bacc("TRN2", target_bir_lowering=False, debug=False)

    specs = {
        "xT": ([128, DCH, T], F32),
        "bmT": ([128, DCH, NB], BF16),
        "lq_w": ([128, DCH, D], BF16),
        "lk_w": ([128, DCH, D], BF16),
        "lv_w": ([128, DCH, D], BF16),
        "lo_w": ([64, H, D], BF16),
        "gq_w": ([128, DCH, D], BF16),
        "gkv_w": ([128, DCH, 1280], BF16),
        "go_w": ([64, H, D], BF16),
        "f1_w": ([128, DCH, HID], BF16),
        "f2_w": ([128, HCH, D], BF16),
        "bias_pack": ([128, 5, 7], F32),
        "f1_bp": ([128, HCH], F32),
        "vb65": ([65, 2, H], F32),
        "maskT": ([128, 128], BF16),
    }
    io = {}
    for name, (shape, dt_) in specs.items():
        io[name] = nc.dram_tensor(name, shape, dt_, kind="ExternalInput").ap()
    io["out"] = nc.dram_tensor("out", [128, DCH, T], F32,
                               kind="ExternalOutput").ap()

    with tile.TileContext(nc) as tc:
        _emit(nc, tc, io)
    nc.compile()
    _CACHE["nc"] = nc
    return nc


def _fm(a):
    Tn, Dn = a.shape
    return np.ascontiguousarray(
        a.T.reshape(Dn // 128, 128, Tn).transpose(1, 0, 2))


def _wfm(w):
    din, dout = w.shape
    return np.ascontiguousarray(
        w.reshape(din // 128, 128, dout).transpose(1, 0, 2))


def _whead(w):
    return np.ascontiguousarray(w.reshape(H, DH, -1).transpose(1, 0, 2))


def _bp(b):
    return np.ascontiguousarray(b.reshape(-1, 128).T)


def prepare_in_maps(inputs):
    f32 = lambda a: np.asarray(a, dtype=np.float32)

    x = f32(inputs["x"]).reshape(2 * 4096, D)
    bias_pack = np.ascontiguousarray(np.stack(
        [_bp(f32(inputs["lq_b"])), _bp(f32(inputs["lk_b"])),
         _bp(f32(inputs["lo_b"])), _bp(f32(inputs["gq_b"])),
         _bp(f32(inputs["gkv_b"])[:D]), _bp(f32(inputs["go_b"])),
         _bp(f32(inputs["f2_b"]))],
        axis=2))

    vb65 = np.zeros((65, 2, H), np.float32)
    vb65[:64, 0, :] = f32(inputs["lv_b"]).reshape(H, DH).T
    vb65[:64, 1, :] = f32(inputs["gkv_b"])[D:].reshape(H, DH).T

    maskT = (np.arange(128)[:, None] <= np.arange(128)[None, :]).astype(BF)

    shared = {
        "bmT": _fm(f32(inputs["bucket_matrix"])).astype(BF),
        "lq_w": _wfm(f32(inputs["lq_w"])).astype(BF),
        "lk_w": _wfm(f32(inputs["lk_w"])).astype(BF),
        "lv_w": _wfm(f32(inputs["lv_w"])).astype(BF),
        "lo_w": _whead(f32(inputs["lo_w"])).astype(BF),
        "gq_w": _wfm(f32(inputs["gq_w"])).astype(BF),
        "gkv_w": _wfm(f32(inputs["gkv_w"])).astype(BF),
        "go_w": _whead(f32(inputs["go_w"])).astype(BF),
        "f1_w": _wfm(f32(inputs["f1_w"])).astype(BF),
        "f2_w": _wfm(f32(inputs["f2_w"])).astype(BF),
        "bias_pack": bias_pack,
        "f1_bp": _bp(f32(inputs["f1_b"])),
        "vb65": vb65,
        "maskT": maskT,
    }
    in_maps = []
    for c in range(N_CORES):
        m = dict(shared)
        m["xT"] = _fm(x[c * T:(c + 1) * T])
        in_maps.append(m)
    return in_maps


def assemble(results):
    out = np.empty((2 * 4096, D), np.float32)
    for c in range(N_CORES):
        o = np.asarray(results[c]["out"])
        out[c * T:(c + 1) * T] = o.transpose(2, 1, 0).reshape(T, D)
    return out.reshape(2, 4096, D)


def kernel(**inputs):
    nc = _build()
    in_maps = prepare_in_maps(inputs)
    res = run_bass_kernel_spmd(nc, in_maps, list(range(N_CORES)))
    return assemble(res.results)


# revision 19
# speedup vs baseline: 1.5904x; 1.0261x over previous
import sys

sys.path.insert(0, "/opt/trn_rl_repo")

import numpy as np
import ml_dtypes

import concourse.bass as bass
import concourse.mybir as mybir
import concourse.tile as tile
from concourse import bacc
from concourse.bass_utils import run_bass_kernel_spmd

F32 = mybir.dt.float32
BF16 = mybir.dt.bfloat16
AF = mybir.ActivationFunctionType
OP = mybir.AluOpType
BF = ml_dtypes.bfloat16

D = 640
DCH = 5
H = 10
DH = 64
T = 1024
W = 128
NW = T // W
NB = 4096
NBC = NB // 128
HID = 4 * D
HCH = HID // 128
N_CORES = 8
EPS = 1e-5
SCALE = 1.0 / 8.0


def _emit(nc, tc, io):
    from contextlib import ExitStack

    with ExitStack() as ctx:
        const = ctx.enter_context(tc.tile_pool(name="const", bufs=1))
        xchain = ctx.enter_context(tc.tile_pool(name="xchain", bufs=2))
        smse = ctx.enter_context(tc.tile_pool(name="smse", bufs=2))
        smln = ctx.enter_context(tc.tile_pool(name="smln", bufs=1))
        lnp = ctx.enter_context(tc.tile_pool(name="lnp", bufs=1))
        psum = ctx.enter_context(tc.tile_pool(name="psum", bufs=2, space="PSUM"))

        maskT = const.tile([128, 128], BF16, tag="maskT")
        nc.sync.dma_start(maskT[:], io["maskT"][:])
        biases = const.tile([128, 5, 7], F32, tag="biases")
        nc.sync.dma_start(biases[:], io["bias_pack"][:])
        lq_bp = lambda m: biases[:, m, 0:1]
        lk_bp = lambda m: biases[:, m, 1:2]
        lo_bp = lambda m: biases[:, m, 2:3]
        gq_bp = lambda m: biases[:, m, 3:4]
        gk_bp = lambda m: biases[:, m, 4:5]
        go_bp = lambda m: biases[:, m, 5:6]
        f2_bp = lambda m: biases[:, m, 6:7]
        f1_bp = const.tile([128, HCH], F32, tag="f1_bp")
        nc.sync.dma_start(f1_bp[:], io["f1_bp"][:])
        vb65 = const.tile([65, 2, H], F32, tag="vb65")
        nc.sync.dma_start(vb65[:], io["vb65"][:])
        ones65 = const.tile([65, 64], F32, tag="ones65")
        nc.vector.memset(ones65[:], 1.0)
        ones_p0 = const.tile([1, 128], F32, tag="ones_p0")
        nc.vector.memset(ones_p0[:], 1.0)
        ones128b = const.tile([128, 1], BF16, tag="ones128b")
        nc.vector.memset(ones128b[:], 1.0)
        eps_t = const.tile([1, 1], F32, tag="eps")
        nc.vector.memset(eps_t[:], EPS)

        def make_loader(pool):
            def load_w(name, shape, dt_):
                t = pool.tile(shape, dt_, tag="w640", name="wt")
                nc.sync.dma_start(t[:], io[name][:])
                return t
            return load_w

        def proj_fm(x_sb, w_sb, out_sb, bias_fn):
            for sc in range(2):
                for m in range(DCH):
                    pt = psum.tile([128, 512], F32, tag="pp")
                    for k in range(DCH):
                        nc.tensor.matmul(
                            pt[:],
                            w_sb[:, k, m * 128:(m + 1) * 128],
                            x_sb[:, k, sc * 512:(sc + 1) * 512],
                            start=(k == 0),
                            stop=(k == DCH - 1),
                        )
                    nc.vector.tensor_scalar_add(
                        out_sb[:, m, sc * 512:(sc + 1) * 512], pt[:], bias_fn(m)
                    )

        def ln_to(x_in, x_out):
            for sc in range(2):
                sl = slice(sc * 512, (sc + 1) * 512)
                xb = lnp.tile([128, DCH, 512], BF16, tag="ln_xb")
                xq = lnp.tile([128, DCH, 512], BF16, tag="ln_xq")
                for k in range(DCH):
                    nc.vector.tensor_copy(xb[:, k, :], x_in[:, k, sl])
                    nc.vector.tensor_tensor(
                        xq[:, k, :], x_in[:, k, sl], x_in[:, k, sl], OP.mult
                    )
                pm = psum.tile([1, 512], F32, tag="pp")
                pq = psum.tile([1, 512], F32, tag="pp")
                for k in range(DCH):
                    nc.tensor.matmul(pm[:], ones128b[:], xb[:, k, :],
                                     start=(k == 0), stop=(k == DCH - 1))
                for k in range(DCH):
                    nc.tensor.matmul(pq[:], ones128b[:], xq[:, k, :],
                                     start=(k == 0), stop=(k == DCH - 1))
                mean = smln.tile([1, 512], F32, tag="ln_mean")
                var = smln.tile([1, 512], F32, tag="ln_var")
                rstd = smln.tile([1, 512], F32, tag="ln_rstd")
                cc = smln.tile([1, 512], F32, tag="ln_c")
                nc.vector.tensor_scalar_mul(mean[:], pm[:], 1.0 / D)
                nc.vector.tensor_scalar_mul(var[:], pq[:], 1.0 / D)
                nc.vector.tensor_tensor(cc[:], mean[:], mean[:], OP.mult)
                nc.vector.tensor_tensor(var[:], var[:], cc[:], OP.subtract)
                nc.scalar.activation(rstd[:], var[:], AF.Sqrt, bias=eps_t[:])
                nc.vector.reciprocal(rstd[:], rstd[:])
                nc.vector.tensor_tensor(cc[:], mean[:], rstd[:], OP.mult)
                nc.vector.tensor_scalar_mul(cc[:], cc[:], -1.0)
                pa = psum.tile([128, 512], F32, tag="pp")
                pc = psum.tile([128, 512], F32, tag="pp")
                nc.tensor.matmul(pa[:], ones_p0[:], rstd[:],
                                 start=True, stop=True)
                nc.tensor.matmul(pc[:], ones_p0[:], cc[:],
                                 start=True, stop=True)
                for k in range(DCH):
                    nc.vector.tensor_tensor(x_out[:, k, sl], x_in[:, k, sl],
                                            pa[:], OP.mult)
                    nc.vector.tensor_tensor(x_out[:, k, sl], x_out[:, k, sl],
                                            pc[:], OP.add)

        def attn_finish(pacc, pj, sc, vb_base, pool, ow_sb, xr_sb):
            sl = slice(sc * 512, (sc + 1) * 512)
            ats = []
            for i in range(2):
                ps_a = pacc[i]
                se = smse.tile([65, 512], F32, tag="sumexp")
                nc.vector.tensor_copy(se[64:65, :], ps_a[64:65, :])
                nc.vector.reciprocal(se[64:65, :], se[64:65, :])
                pr = psum.tile([64, 512], F32, tag="pp")
                nc.tensor.matmul(pr[:], ones65[64:65, :],
                                 se[64:65, :], start=True, stop=True)
                prs = smse.tile([64, 512], F32, tag="prs")
                nc.vector.tensor_copy(prs[:], pr[:])
                at = smse.tile([64, 512], BF16, tag=f"attn{i}", name=f"at{i}")
                nc.vector.tensor_tensor(at[:], ps_a[0:64, :], prs[:], OP.mult)
                nc.vector.tensor_scalar_add(
                    at[:], at[:], vb_base[0:64, 2 * pj + i:2 * pj + i + 1])
                ats.append(at)
            for m in range(DCH):
                po = psum.tile([128, 512], F32, tag="pp")
                for i in range(2):
                    h = 2 * pj + i
                    nc.tensor.matmul(po[:],
                                     ow_sb[:, h, m * 128:(m + 1) * 128],
                                     ats[i][:],
                                     start=(i == 0), stop=(i == 1))
                nc.vector.tensor_tensor(xr_sb[:, m, sl], po[:],
                                        xr_sb[:, m, sl], OP.add)

        x_sb = xchain.tile([128, DCH, T], F32, tag="x")
        nc.sync.dma_start(x_sb[:], io["xT"][:])

        with tc.tile_pool(name="pL", bufs=1) as pL, \
             tc.tile_pool(name="pLw", bufs=2) as pLw, \
             tc.tile_pool(name="pLe", bufs=6) as pLe, \
             tc.tile_pool(name="psumL", bufs=2, space="PSUM") as psumL, \
             tc.tile_pool(name="psaccL", bufs=1, space="PSUM") as psaccL:
            load_w = make_loader(pLw)
            q_sb = pL.tile([128, DCH, T], BF16, tag="q")
            k_sb = pL.tile([128, DCH, T], BF16, tag="k")
            v_sb = pL.tile([128, NW, H, 65], BF16, tag="v")
            xb16 = pL.tile([128, DCH, T], BF16, tag="xb16")
            for k in range(DCH):
                nc.vector.tensor_copy(xb16[:, k, :], x_sb[:, k, :])

            lw = load_w("lq_w", [128, DCH, D], BF16)
            proj_fm(xb16, lw, q_sb, lq_bp)
            lw = load_w("lk_w", [128, DCH, D], BF16)
            proj_fm(xb16, lw, k_sb, lk_bp)

            nc.vector.memset(v_sb[:, :, :, 64:65], 1.0)
            lw = load_w("lv_w", [128, DCH, D], BF16)
            for st in range(NW):
                ssl = slice(st * 128, (st + 1) * 128)
                for j in range(2):
                    pt = psum.tile([128, 320], F32, tag="pp")
                    for k in range(DCH):
                        nc.tensor.matmul(
                            pt[:],
                            xb16[:, k, ssl],
                            lw[:, k, j * 320:(j + 1) * 320],
                            start=(k == 0),
                            stop=(k == DCH - 1),
                        )
                    nc.vector.tensor_copy(
                        v_sb[:, st, j * 5:(j + 1) * 5, 0:64],
                        pt[:].rearrange("p (h d) -> p h d", d=64),
                    )

            for m in range(DCH):
                nc.vector.tensor_scalar_add(x_sb[:, m, :], x_sb[:, m, :],
                                            lo_bp(m))
            low = load_w("lo_w", [64, H, D], BF16)

            for pj in range(DCH):
                for sc in range(2):
                    pacc = [
                        psaccL.tile([65, 512], F32, tag=f"pacc{i}",
                                    name=f"pacc{i}") for i in range(2)
                    ]
                    ps_s = psumL.tile([128, 2, 4, 128], F32, tag="psl",
                                      name="psl")
                    for i in range(2):
                        hp = slice(i * 64, (i + 1) * 64)
                        for wi in range(4):
                            w = sc * 4 + wi
                            wsl = slice(w * 128, (w + 1) * 128)
                            nc.tensor.matmul(ps_s[:, i, wi, :],
                                             k_sb[hp, pj, wsl],
                                             q_sb[hp, pj, wsl],
                                             start=True, stop=True)
                    ex = pLe.tile([128, 2, 4, 128], BF16, tag="expl",
                                  name="ex")
                    nc.scalar.activation(ex[:], ps_s[:], AF.Exp, scale=SCALE)
                    nc.vector.tensor_tensor(
                        ex[:], ex[:],
                        maskT[:, None, None, :].to_broadcast((128, 2, 4, 128)),
                        OP.mult)
                    for i in range(2):
                        h = 2 * pj + i
                        for wi in range(4):
                            nc.tensor.matmul(
                                pacc[i][:, wi * 128:(wi + 1) * 128],
                                v_sb[:, sc * 4 + wi, h, :], ex[:, i, wi, :],
                                start=True, stop=True,
                            )
                    attn_finish(pacc, pj, sc, vb65[:, 0, :], pLe, low, x_sb)

        x1 = xchain.tile([128, DCH, T], F32, tag="x")
        ln_to(x_sb, x1)

        with tc.tile_pool(name="pG", bufs=1) as pG, \
             tc.tile_pool(name="pGw", bufs=1) as pGw, \
             tc.tile_pool(name="pGb", bufs=2) as pGb, \
             tc.tile_pool(name="pGe", bufs=4) as pGe, \
             tc.tile_pool(name="psumG", bufs=2, space="PSUM") as psumG, \
             tc.tile_pool(name="psaccG", bufs=1, space="PSUM") as psaccG:
            load_w = make_loader(pGw)
            q2 = pG.tile([128, DCH, T], BF16, tag="q2")
            kF = pG.tile([128, DCH, NB], BF16, tag="kF")
            vG = pG.tile([128, NBC, H, 65], BF16, tag="vG")

            x1b = pG.tile([128, DCH, T], BF16, tag="x1b")
            for k in range(DCH):
                nc.vector.tensor_copy(x1b[:, k, :], x1[:, k, :])
            gw = load_w("gq_w", [128, DCH, D], BF16)
            proj_fm(x1b, gw, q2, gq_bp)

            nc.vector.memset(vG[:, :, :, 64:65], 1.0)
            gkv = load_w("gkv_w", [128, DCH, 1280], BF16)
            for ng in range(8):
                nsl = slice(ng * 512, (ng + 1) * 512)
                bm = pGb.tile([128, DCH, 512], BF16, tag="bm")
                nc.sync.dma_start(bm[:], io["bmT"][:, :, nsl])
                for m in range(DCH):
                    pt = psum.tile([128, 512], F32, tag="pp")
                    for k in range(DCH):
                        nc.tensor.matmul(pt[:], gkv[:, k, m * 128:(m + 1) * 128],
                                         bm[:, k, :],
                                         start=(k == 0), stop=(k == DCH - 1))
                    nc.vector.tensor_scalar_add(kF[:, m, nsl], pt[:], gk_bp(m))
                for nb in range(4):
                    n = ng * 4 + nb
                    bsl = slice(nb * 128, (nb + 1) * 128)
                    for j in range(2):
                        pt = psum.tile([128, 320], F32, tag="pp")
                        for k in range(DCH):
                            nc.tensor.matmul(
                                pt[:], bm[:, k, bsl],
                                gkv[:, k, 640 + j * 320:640 + (j + 1) * 320],
                                start=(k == 0), stop=(k == DCH - 1))
                        nc.vector.tensor_copy(
                            vG[:, n, j * 5:(j + 1) * 5, 0:64],
                            pt[:].rearrange("p (h d) -> p h d", d=64),
                        )

            for m in range(DCH):
                nc.vector.tensor_scalar_add(x1[:, m, :], x1[:, m, :], go_bp(m))
            gow = load_w("go_w", [64, H, D], BF16)

            for pj in range(DCH):
                for sc in range(2):
                    ssl = slice(sc * 512, (sc + 1) * 512)
                    pacc = [
                        psaccG.tile([65, 512], F32, tag=f"pacc{i}",
                                    name=f"pacc{i}") for i in range(2)
                    ]
                    for n in range(NBC):
                        nsl = slice(n * 128, (n + 1) * 128)
                        ps_s = psumG.tile([128, 2, 512], F32, tag="psg",
                                          name="psg")
                        for i in range(2):
                            hp = slice(i * 64, (i + 1) * 64)
                            nc.tensor.matmul(ps_s[:, i, :], kF[hp, pj, nsl],
                                             q2[hp, pj, ssl],
                                             start=True, stop=True)
                        ex = pGe.tile([128, 2, 512], BF16, tag="expg",
                                      name="ex")
                        nc.scalar.activation(ex[:], ps_s[:], AF.Exp,
                                             scale=SCALE)
                        for i in range(2):
                            h = 2 * pj + i
                            nc.tensor.matmul(pacc[i][:], vG[:, n, h, :],
                                             ex[:, i, :],
                                             start=(n == 0),
                                             stop=(n == NBC - 1))
                    attn_finish(pacc, pj, sc, vb65[:, 1, :], pGe, gow, x1)

        x2 = xchain.tile([128, DCH, T], F32, tag="x")
        ln_to(x1, x2)

        with tc.tile_pool(name="pF", bufs=1) as pF:
            f1w = pF.tile([128, DCH, HID], BF16, tag="f1w")
            nc.sync.dma_start(f1w[:], io["f1_w"][:])
            f2w = pF.tile([128, HCH, D], BF16, tag="f2w")
            nc.sync.dma_start(f2w[:], io["f2_w"][:])
            x2b = pF.tile([128, DCH, T], BF16, tag="x2b")
            for k in range(DCH):
                nc.vector.tensor_copy(x2b[:, k, :], x2[:, k, :])
            xr3 = x2
            for m in range(DCH):
                nc.vector.tensor_scalar_add(xr3[:, m, :], xr3[:, m, :], f2_bp(m))
            for sc in range(2):
                sl = slice(sc * 512, (sc + 1) * 512)
                hsb = pF.tile([128, HCH, 512], BF16, tag=f"h{sc}")
                for mh in range(HCH):
                    pt = psum.tile([128, 512], F32, tag="pp")
                    for k in range(DCH):
                        nc.tensor.matmul(pt[:],
                                         f1w[:, k, mh * 128:(mh + 1) * 128],
                                         x2b[:, k, sl],
                                         start=(k == 0), stop=(k == DCH - 1))
                    nc.scalar.activation(hsb[:, mh, :], pt[:], AF.Gelu,
                                         bias=f1_bp[:, mh:mh + 1])
                for m in range(DCH):
                    pt = psum.tile([128, 512], F32, tag="pp")
                    for k in range(HCH):
                        nc.tensor.matmul(pt[:],
                                         f2w[:, k, m * 128:(m + 1) * 128],
                                         hsb[:, k, :],
                                         start=(k == 0), stop=(k == HCH - 1))
                    nc.vector.tensor_tensor(xr3[:, m, sl], pt[:], xr3[:, m, sl],
                                            OP.add)

        out_sb = xchain.tile([128, DCH, T], F32, tag="x")
        ln_to(xr3, out_sb)
        nc.sync.dma_start(io["out"][:], out_sb[:])



_CACHE = {}


def _build():
    if "nc" in _CACHE:
        return _CACHE["nc"]
    nc = bacc.Bacc("TRN2", target_bir_lowering=False, debug=False)

    specs = {
        "xT": ([128, DCH, T], F32),
        "bmT": ([128, DCH, NB], BF16),
        "lq_w": ([128, DCH, D], BF16),
        "lk_w": ([128, DCH, D], BF16),
        "lv_w": ([128, DCH, D], BF16),
        "lo_w": ([64, H, D], BF16),
        "gq_w": ([128, DCH, D], BF16),
        "gkv_w": ([128, DCH, 1280], BF16),
        "go_w": ([64, H, D], BF16),
        "f1_w": ([128, DCH, HID], BF16),
        "f2_w": ([128, HCH, D], BF16),
        "bias_pack": ([128, 5, 7], F32),
        "f1_bp": ([128, HCH], F32),
        "vb65": ([65, 2, H], F32),
        "maskT": ([128, 128], BF16),
    }
    io = {}
    for name, (shape, dt_) in specs.items():
        io[name] = nc.dram_tensor(name, shape, dt_, kind="ExternalInput").ap()
    io["out"] = nc.dram_tensor("out", [128, DCH, T], F32,
                               kind="ExternalOutput").ap()

    with tile.TileContext(nc) as tc:
        _emit(nc, tc, io)
    nc.compile()
    _CACHE["nc"] = nc
    return nc


def _fm(a):
    Tn, Dn = a.shape
    return np.ascontiguousarray(
        a.T.reshape(Dn // 128, 128, Tn).transpose(1, 0, 2))


def _wfm(w):
    din, dout = w.shape
    return np.ascontiguousarray(
        w.reshape(din // 128, 128, dout).transpose(1, 0, 2))


def _whead(w):
    return np.ascontiguousarray(w.reshape(H, DH, -1).transpose(1, 0, 2))


def _bp(b):
    return np.ascontiguousarray(b.reshape(-1, 128).T)


def prepare_in_maps(inputs):
    f32 = lambda a: np.asarray(a, dtype=np.float32)

    x = f32(inputs["x"]).reshape(2 * 4096, D)
    bias_pack = np.ascontiguousarray(np.stack(
        [_bp(f32(inputs["lq_b"])), _bp(f32(inputs["lk_b"])),
         _bp(f32(inputs["lo_b"])), _bp(f32(inputs["gq_b"])),
         _bp(f32(inputs["gkv_b"])[:D]), _bp(f32(inputs["go_b"])),
         _bp(f32(inputs["f2_b"]))],
        axis=2))

    vb65 = np.zeros((65, 2, H), np.float32)
    vb65[:64, 0, :] = f32(inputs["lv_b"]).reshape(H, DH).T
    vb65[:64, 1, :] = f32(inputs["gkv_b"])[D:].reshape(H, DH).T

    maskT = (np.arange(128)[:, None] <= np.arange(128)[None, :]).astype(BF)

    shared = {
        "bmT": _fm(f32(inputs["bucket_matrix"])).astype(BF),
        "lq_w": _wfm(f32(inputs["lq_w"])).astype(BF),
        "lk_w": _wfm(f32(inputs["lk_w"])).astype(BF),
        "lv_w": _wfm(f32(inputs["lv_w"])).astype(BF),
        "lo_w": _whead(f32(inputs["lo_w"])).astype(BF),
        "gq_w": _wfm(f32(inputs["gq_w"])).astype(BF),
        "gkv_w": _wfm(f32(inputs["gkv_w"])).astype(BF),
        "go_w": _whead(f32(inputs["go_w"])).astype(BF),
        "f1_w": _wfm(f32(inputs["f1_w"])).astype(BF),
        "f2_w": _wfm(f32(inputs["f2_w"])).astype(BF),
        "bias_pack": bias_pack,
        "f1_bp": _bp(f32(inputs["f1_b"])),
        "vb65": vb65,
        "maskT": maskT,
    }
    in_maps = []
    for c in range(N_CORES):
        m = dict(shared)
        m["xT"] = _fm(x[c * T:(c + 1) * T])
        in_maps.append(m)
    return in_maps


def assemble(results):
    out = np.empty((2 * 4096, D), np.float32)
    for c in range(N_CORES):
        o = np.asarray(results[c]["out"])
        out[c * T:(c + 1) * T] = o.transpose(2, 1, 0).reshape(T, D)
    return out.reshape(2, 4096, D)


def kernel(**inputs):
    nc = _build()
    in_maps = prepare_in_maps(inputs)
    res = run_bass_kernel_spmd(nc, in_maps, list(range(N_CORES)))
    return assemble(res.results)
